# revision 18
# baseline (speedup 1.0000x reference)
"""Trainium2 Bass kernel for softmax RGB blend (pytorch3d NoLightShader).

Full inputs (N=8, H=512, W=512, K=8) are sharded batch-wise across 8
NeuronCores (one batch image per core); the blend is purely per-pixel so no
cross-core communication is needed.

Math per pixel (K faces):
    mask_k  = pix_to_face_k >= 0
    prob_k  = sigmoid(-dists_k / SIGMA) * mask_k
    alpha   = 1 - prod_k(1 - prob_k)        (via exp(sum ln(1 - prob_k)))
    z_k     = (ZFAR - zbuf_k) / (ZFAR - ZNEAR) * mask_k
    zmax    = max_k z_k                     (EPS clamp dropped: only matters
                                             for all-masked pixels, where the
                                             result is unchanged)
    w_k     = prob_k * exp((z_k - zmax) / GAMMA)
    delta   = exp((EPS - zmax) / GAMMA)
    denom   = sum_k w_k + delta
    rgb_c   = (sum_k w_k * color_kc + delta) / denom    (background = 1,1,1)
    out     = [rgb, alpha]

Raw-bass pipeline (Tile's multi-wait instructions don't compile on this
walrus, so waits are explicit single-sem instructions):
    SP  (sync) : HWDGE DMAs in/out, double-buffered input slots
    ACT (scalar): sigmoid, z-linearize, ln(1-prob), exp(zd/g), delta, prod(q)
    DVE (vector): mask, mask applies, the four K-reductions, w, w*c,
                  denom, reciprocal, rgb/alpha finalize
Per-tile op indices give deterministic semaphore thresholds; see marks below.
"""

import sys
from contextlib import ExitStack

import numpy as np

if "/opt/trn_rl_repo" not in sys.path:
    sys.path.insert(0, "/opt/trn_rl_repo")

SIGMA = 1e-4
GAMMA = 1e-4
ZNEAR = 1.0
ZFAR = 100.0
EPS = 1e-10

P = 128
K = 8
N_CORES = 8

# per-tile op counts (sem increments per tile per engine)
N_ACT = 9   # sig, zlin, lnq, ex, delta, prodq, lnd, rcp, alpha
N_DVE = 10  # prob, zinv, zmax, qsum, wsum, denom, wc, csum, t3, rgb
N_GP = 2    # zd, w


def build_program(rows, T):
    import concourse.bass as bass
    from concourse import mybir

    dt = mybir.dt
    f32 = dt.float32
    Alu = mybir.AluOpType
    Act = mybir.ActivationFunctionType
    Ax = mybir.AxisListType

    assert rows % T == 0
    n = rows // T
    TK = T * K

    nc = bass.Bass()

    zb_d = nc.dram_tensor("zbuf", [P, rows * K], f32, kind="ExternalInput")
    ds_d = nc.dram_tensor("dists", [P, rows * K], f32, kind="ExternalInput")
    pf_d = nc.dram_tensor("pix_to_face", [P, rows * K], dt.int32, kind="ExternalInput")
    pc_d = nc.dram_tensor("pixel_colors", [P, rows * K * 3], f32, kind="ExternalInput")
    out_d = nc.dram_tensor("out", [P, rows * 4], f32, kind="ExternalOutput")

    # const AP for the delta bias (EPS/GAMMA); framework pre-registers 0.0/1.0
    cbias = nc.alloc_sbuf_tensor("c_epsg", [P, 1], f32)
    nc.gpsimd.memset(cbias.ap(), EPS / GAMMA)
    nc.const_aps.aps[(f32, EPS / GAMMA)] = cbias.ap()
    nc.all_engine_barrier()

    with ExitStack() as ctx:
        sb = lambda name, w: ctx.enter_context(nc.sbuf_tensor(name, [P, w], f32))
        zb = [sb(f"zb{j}", TK) for j in range(2)]
        ds = [sb(f"ds{j}", TK) for j in range(2)]
        pf = [
            ctx.enter_context(nc.sbuf_tensor(f"pf{j}", [P, TK], dt.int32))
            for j in range(2)
        ]
        col = [sb(f"col{j}", TK * 3) for j in range(2)]
        ot = [sb(f"ot{j}", T * 4) for j in range(2)]
        sig = sb("sig", TK)      # becomes prob in place
        zlin = sb("zlin", TK)    # becomes zinv in place
        lnq = sb("lnq", TK)
        zd = sb("zd", TK)        # becomes ex in place (ACT)
        w = sb("w", TK)
        wc = sb("wc", TK * 3)
        zmax = sb("zmax", T)
        qsum = sb("qsum", T)
        wsum = sb("wsum", T)
        csum = sb("csum", T * 3)
        delta = [sb(f"delta{j}", T) for j in range(2)]   # cross-iter lifetime
        prodq = [sb(f"prodq{j}", T) for j in range(2)]   # cross-iter lifetime
        denom = sb("denom", T)
        rcp = sb("rcp", T)
        t3 = sb("t3", T * 3)

        s_in = [
            ctx.enter_context(nc.semaphore("s_in0")),
            ctx.enter_context(nc.semaphore("s_in1")),
        ]
        s_out = [
            ctx.enter_context(nc.semaphore("s_out0")),
            ctx.enter_context(nc.semaphore("s_out1")),
        ]
        s_act = ctx.enter_context(nc.semaphore("s_act"))
        s_dve = ctx.enter_context(nc.semaphore("s_dve"))
        s_gp = ctx.enter_context(nc.semaphore("s_gp"))

        # ---- two-pass schedule: pass 1 records per-op sem values (marks),
        # ---- pass 2 emits with waits resolved from the marks.
        marks = {}

        def mk(engkey, name, t, ctr):
            marks[(engkey, name, t)] = ctr

        def sched_sp(sp):
            for i in range(n + 1):
                if i < n:
                    j = i % 2
                    if sp is not None:
                        if i >= 2:
                            sp.wait_ge(s_act, marks[("a", "zlin", i - 2)])
                            sp.wait_ge(s_dve, marks[("d", "wc", i - 2)])
                        sp.dma_start(out=zb[j][:], in_=zb_d[:, bass.ts(i, TK)]
                                     ).then_inc(s_in[j], 16)
                        sp.dma_start(out=ds[j][:], in_=ds_d[:, bass.ts(i, TK)]
                                     ).then_inc(s_in[j], 16)
                        sp.dma_start(out=pf[j][:], in_=pf_d[:, bass.ts(i, TK)]
                                     ).then_inc(s_in[j], 16)
                        sp.dma_start(out=col[j][:], in_=pc_d[:, bass.ts(i, TK * 3)]
                                     ).then_inc(s_in[j], 16)
                if i >= 1:
                    t = i - 1
                    if sp is not None:
                        sp.wait_ge(s_dve, marks[("d", "rgb", t)])
                        sp.wait_ge(s_act, marks[("a", "alpha", t)])
                        sp.dma_start(
                            out=out_d[:, bass.ts(t, T * 4)], in_=ot[t % 2][:]
                        ).then_inc(s_out[t % 2], 16)
            if sp is not None:
                sp.wait_ge(s_out[0], 16 * ((n + 1) // 2))
                sp.wait_ge(s_out[1], 16 * (n // 2))

        def sched_act(act):
            c = 0
            for i in range(n + 1):
                if i < n:
                    j = i % 2
                    if act is not None:
                        act.wait_ge(s_in[j], 64 * (i // 2 + 1))
                        if i >= 1:
                            act.wait_ge(s_gp, marks[("g", "w", i - 1)])
                        act.activation(sig[:], ds[j][:], Act.Sigmoid,
                                       scale=-1.0 / SIGMA).then_inc(s_act, 1)
                    c += 1; mk("a", "sig", i, c)
                    if act is not None:
                        act.activation(
                            zlin[:], zb[j][:], Act.Copy,
                            bias=ZFAR / (ZFAR - ZNEAR),
                            scale=-1.0 / (ZFAR - ZNEAR),
                        ).then_inc(s_act, 1)
                    c += 1; mk("a", "zlin", i, c)
                    if act is not None:
                        act.wait_ge(s_dve, marks[("d", "prob", i)])
                        act.activation(lnq[:], sig[:], Act.Ln, bias=1.0,
                                       scale=-1.0).then_inc(s_act, 1)
                    c += 1; mk("a", "lnq", i, c)
                    if act is not None:
                        act.wait_ge(s_gp, marks[("g", "zd", i)])
                        act.activation(zd[:], zd[:], Act.Exp,
                                       scale=1.0 / GAMMA).then_inc(s_act, 1)
                    c += 1; mk("a", "ex", i, c)
                    if act is not None:
                        act.activation(
                            delta[i % 2][:], zmax[:], Act.Exp,
                            bias=EPS / GAMMA, scale=-1.0 / GAMMA,
                        ).then_inc(s_act, 1)
                    c += 1; mk("a", "delta", i, c)
                    if act is not None:
                        act.wait_ge(s_dve, marks[("d", "qsum", i)])
                        act.activation(prodq[i % 2][:], qsum[:], Act.Exp
                                       ).then_inc(s_act, 1)
                    c += 1; mk("a", "prodq", i, c)
                if i >= 1:
                    t = i - 1
                    if act is not None:
                        act.wait_ge(s_dve, marks[("d", "denom", t)])
                        act.activation(denom[:], denom[:], Act.Ln
                                       ).then_inc(s_act, 1)
                    c += 1; mk("a", "lnd", t, c)
                    if act is not None:
                        act.drain()
                        act.activation(rcp[:], denom[:], Act.Exp, scale=-1.0
                                       ).then_inc(s_act, 1)
                    c += 1; mk("a", "rcp", t, c)
                    if act is not None:
                        if t >= 2:
                            act.wait_ge(s_out[t % 2], 16 * ((t - 2) // 2 + 1))
                        ot_v = ot[t % 2][:].rearrange("p (t q) -> p t q", q=4)
                        act.activation(
                            ot_v[:, :, 3:4], prodq[t % 2][:].unsqueeze(2),
                            Act.Copy, bias=1.0, scale=-1.0,
                        ).then_inc(s_act, 1)
                    c += 1; mk("a", "alpha", t, c)

        def sched_dve(dve):
            c = 0
            for i in range(n + 1):
                if i < n:
                    j = i % 2
                    if dve is not None:
                        dve.wait_ge(s_in[j], 64 * (i // 2 + 1))
                        dve.wait_ge(s_act, marks[("a", "zlin", i)])
                        dve.scalar_tensor_tensor(
                            out=sig[:], in0=pf[j][:], scalar=0.0, in1=sig[:],
                            op0=Alu.is_ge, op1=Alu.mult,
                        ).then_inc(s_dve, 1)
                        dve.drain()
                    c += 1; mk("d", "prob", i, c)
                    if dve is not None:
                        dve.scalar_tensor_tensor(
                            out=zlin[:], in0=pf[j][:], scalar=0.0, in1=zlin[:],
                            op0=Alu.is_ge, op1=Alu.mult,
                        ).then_inc(s_dve, 1)
                        dve.drain()
                    c += 1; mk("d", "zinv", i, c)
                    if dve is not None:
                        dve.tensor_reduce(
                            out=zmax[:],
                            in_=zlin[:].rearrange("p (t k) -> p t k", k=K),
                            op=Alu.max, axis=Ax.X,
                        ).then_inc(s_dve, 1)
                    c += 1; mk("d", "zmax", i, c)
                    if dve is not None:
                        dve.wait_ge(s_act, marks[("a", "lnq", i)])
                        dve.tensor_reduce(
                            out=qsum[:],
                            in_=lnq[:].rearrange("p (t k) -> p t k", k=K),
                            op=Alu.add, axis=Ax.X,
                        ).then_inc(s_dve, 1)
                    c += 1; mk("d", "qsum", i, c)
                if i >= 1:
                    t = i - 1
                    if dve is not None:
                        dve.wait_ge(s_gp, marks[("g", "w", t)])
                        dve.tensor_reduce(
                            out=wsum[:],
                            in_=w[:].rearrange("p (t k) -> p t k", k=K),
                            op=Alu.add, axis=Ax.X,
                        ).then_inc(s_dve, 1)
                        dve.drain()
                    c += 1; mk("d", "wsum", t, c)
                    if dve is not None:
                        dve.wait_ge(s_act, marks[("a", "delta", t)])
                        dve.tensor_tensor(
                            out=denom[:], in0=wsum[:], in1=delta[t % 2][:],
                            op=Alu.add,
                        ).then_inc(s_dve, 1)
                    c += 1; mk("d", "denom", t, c)
                    if dve is not None:
                        wc_v = wc[:].rearrange("p (t c k) -> p t c k", c=3, k=K)
                        dve.tensor_tensor(
                            out=wc_v,
                            in0=w[:].rearrange("p (t k) -> p t k", k=K)
                                .unsqueeze(2).broadcast_to((P, T, 3, K)),
                            in1=col[t % 2][:].rearrange(
                                "p (t k c) -> p t c k", k=K, c=3),
                            op=Alu.mult,
                        ).then_inc(s_dve, 1)
                        dve.drain()
                    c += 1; mk("d", "wc", t, c)
                    if dve is not None:
                        csum_v = csum[:].rearrange("p (t c) -> p t c", c=3)
                        dve.tensor_reduce(
                            out=csum_v, in_=wc_v, op=Alu.add, axis=Ax.X
                        ).then_inc(s_dve, 1)
                        dve.drain()
                    c += 1; mk("d", "csum", t, c)
                    if dve is not None:
                        t3_v = t3[:].rearrange("p (t c) -> p t c", c=3)
                        dve.tensor_tensor(
                            out=t3_v, in0=csum_v,
                            in1=delta[t % 2][:].unsqueeze(2).broadcast_to((P, T, 3)),
                            op=Alu.add,
                        ).then_inc(s_dve, 1)
                        dve.drain()
                    c += 1; mk("d", "t3", t, c)
                    if dve is not None:
                        if t >= 2:
                            dve.wait_ge(s_out[t % 2], 16 * ((t - 2) // 2 + 1))
                        dve.wait_ge(s_act, marks[("a", "rcp", t)])
                        ot_v = ot[t % 2][:].rearrange("p (t q) -> p t q", q=4)
                        dve.tensor_tensor(
                            out=ot_v[:, :, 0:3],
                            in0=t3[:].rearrange("p (t c) -> p t c", c=3),
                            in1=rcp[:].unsqueeze(2).broadcast_to((P, T, 3)),
                            op=Alu.mult,
                        ).then_inc(s_dve, 1)
                    c += 1; mk("d", "rgb", t, c)

        def sched_gp(gp):
            c = 0
            for i in range(n):
                if gp is not None:
                    gp.wait_ge(s_dve, marks[("d", "zmax", i)])
                    gp.tensor_tensor(
                        out=zd[:].rearrange("p (t k) -> p t k", k=K),
                        in0=zlin[:].rearrange("p (t k) -> p t k", k=K),
                        in1=zmax[:].unsqueeze(2).broadcast_to((P, T, K)),
                        op=Alu.subtract,
                    ).then_inc(s_gp, 1)
                c += 1; mk("g", "zd", i, c)
                if gp is not None:
                    gp.wait_ge(s_act, marks[("a", "ex", i)])
                    if i >= 1:
                        gp.wait_ge(s_dve, marks[("d", "wc", i - 1)])
                    gp.tensor_tensor(
                        out=w[:], in0=sig[:], in1=zd[:], op=Alu.mult
                    ).then_inc(s_gp, 1)
                c += 1; mk("g", "w", i, c)

        # pass 1: record marks
        sched_sp(None)
        sched_act(None)
        sched_dve(None)
        sched_gp(None)

        blk = ctx.enter_context(nc.Block())

        @blk.sync
        def _(sp):
            sched_sp(sp)

        @blk.scalar
        def _(act):
            sched_act(act)

        @blk.vector
        def _(dve):
            sched_dve(dve)

        @blk.gpsimd
        def _(gp):
            sched_gp(gp)

    return nc


_CACHE = {}


def _get_program(rows=2048, T=256):
    key = (rows, T)
    if key not in _CACHE:
        _CACHE[key] = build_program(rows, T)
    return _CACHE[key]


def _run(pixel_colors, zbuf, dists, pix_to_face, trace=False):
    from concourse.bass_utils import run_bass_kernel_spmd

    N, H, W, Kk = zbuf.shape
    assert (N, H, W, Kk) == (8, 512, 512, 8), (N, H, W, Kk)
    rows = H * W // P  # 2048

    nc = _get_program(rows=rows, T=256)

    pc = np.ascontiguousarray(np.asarray(pixel_colors, dtype=np.float32))
    zb = np.ascontiguousarray(np.asarray(zbuf, dtype=np.float32))
    ds = np.ascontiguousarray(np.asarray(dists, dtype=np.float32))
    pf = np.ascontiguousarray(np.asarray(pix_to_face, dtype=np.int32))

    in_maps = []
    for i in range(N_CORES):
        in_maps.append(
            {
                "zbuf": zb[i].reshape(P, rows * K),
                "dists": ds[i].reshape(P, rows * K),
                "pix_to_face": pf[i].reshape(P, rows * K),
                "pixel_colors": pc[i].reshape(P, rows * K * 3),
            }
        )

    res = run_bass_kernel_spmd(
        nc, in_maps, core_ids=list(range(N_CORES)), trace=trace
    )
    out = np.stack(
        [res.results[i]["out"].reshape(H, W, 4) for i in range(N_CORES)], axis=0
    )
    return out, res


def kernel(pixel_colors, zbuf, dists, pix_to_face):
    out, _ = _run(pixel_colors, zbuf, dists, pix_to_face, trace=False)
    return out


# revision 20
# speedup vs baseline: 1.0088x; 1.0088x over previous
"""Trainium2 Bass kernel for softmax RGB blend (pytorch3d NoLightShader).

Full inputs (N=8, H=512, W=512, K=8) are sharded batch-wise across 8
NeuronCores (one batch image per core); the blend is purely per-pixel so no
cross-core communication is needed.

Math per pixel (K faces):
    mask_k  = pix_to_face_k >= 0
    prob_k  = sigmoid(-dists_k / SIGMA) * mask_k
    alpha   = 1 - prod_k(1 - prob_k)        (via exp(sum ln(1 - prob_k)))
    z_k     = (ZFAR - zbuf_k) / (ZFAR - ZNEAR) * mask_k
    zmax    = max_k z_k                     (EPS clamp dropped: only matters
                                             for all-masked pixels, where the
                                             result is unchanged)
    w_k     = prob_k * exp((z_k - zmax) / GAMMA)
    delta   = exp((EPS - zmax) / GAMMA)
    denom   = sum_k w_k + delta
    rgb_c   = (sum_k w_k * color_kc + delta) / denom    (background = 1,1,1)
    out     = [rgb, alpha]

Raw-bass pipeline (Tile's multi-wait instructions don't compile on this
walrus, so waits are explicit single-sem instructions):
    SP  (sync) : HWDGE DMAs in/out, double-buffered input slots
    ACT (scalar): sigmoid, z-linearize, ln(1-prob), exp(zd/g), delta, prod(q)
    DVE (vector): mask, mask applies, the four K-reductions, w, w*c,
                  denom, reciprocal, rgb/alpha finalize
Per-tile op indices give deterministic semaphore thresholds; see marks below.
"""

import sys
from contextlib import ExitStack

import numpy as np

if "/opt/trn_rl_repo" not in sys.path:
    sys.path.insert(0, "/opt/trn_rl_repo")

SIGMA = 1e-4
GAMMA = 1e-4
ZNEAR = 1.0
ZFAR = 100.0
EPS = 1e-10

P = 128
K = 8
N_CORES = 8

# per-tile op counts (sem increments per tile per engine)
N_ACT = 9   # sig, zlin, lnq, ex, delta, prodq, lnd, rcp, alpha
N_DVE = 10  # prob, zinv, zmax, qsum, wsum, denom, wc, csum, t3, rgb
N_GP = 2    # zd, w


def build_program(rows, T):
    import concourse.bass as bass
    from concourse import mybir

    dt = mybir.dt
    f32 = dt.float32
    Alu = mybir.AluOpType
    Act = mybir.ActivationFunctionType
    Ax = mybir.AxisListType

    assert rows % T == 0
    n = rows // T
    TK = T * K

    nc = bass.Bass()

    zb_d = nc.dram_tensor("zbuf", [P, rows * K], f32, kind="ExternalInput")
    ds_d = nc.dram_tensor("dists", [P, rows * K], f32, kind="ExternalInput")
    pf_d = nc.dram_tensor("pix_to_face", [P, rows * K], dt.int32, kind="ExternalInput")
    pc_d = nc.dram_tensor("pixel_colors", [P, rows * K * 3], f32, kind="ExternalInput")
    out_d = nc.dram_tensor("out", [P, rows * 4], f32, kind="ExternalOutput")

    # const AP for the delta bias (EPS/GAMMA); framework pre-registers 0.0/1.0
    cbias = nc.alloc_sbuf_tensor("c_epsg", [P, 1], f32)
    nc.gpsimd.memset(cbias.ap(), EPS / GAMMA)
    nc.const_aps.aps[(f32, EPS / GAMMA)] = cbias.ap()
    nc.all_engine_barrier()

    with ExitStack() as ctx:
        sb = lambda name, w: ctx.enter_context(nc.sbuf_tensor(name, [P, w], f32))
        zb = [sb(f"zb{j}", TK) for j in range(2)]
        ds = [sb(f"ds{j}", TK) for j in range(2)]
        pf = [
            ctx.enter_context(nc.sbuf_tensor(f"pf{j}", [P, TK], dt.int32))
            for j in range(2)
        ]
        col = [sb(f"col{j}", TK * 3) for j in range(2)]
        ot = [sb(f"ot{j}", T * 4) for j in range(2)]
        sig = [sb(f"sig{j}", TK) for j in range(2)]    # prob, then w in place
        zlin = [sb(f"zlin{j}", TK) for j in range(2)]  # becomes zinv in place
        lnq = sb("lnq", TK)
        zd = sb("zd", TK)        # becomes ex in place (ACT)
        wc = sb("wc", TK * 3)
        zmax = sb("zmax", T)
        qsum = sb("qsum", T)
        wsum = sb("wsum", T)
        csum = sb("csum", T * 3)
        delta = [sb(f"delta{j}", T) for j in range(2)]   # cross-iter lifetime
        prodq = [sb(f"prodq{j}", T) for j in range(2)]   # cross-iter lifetime
        denom = sb("denom", T)
        rcp = sb("rcp", T)

        s_in = [
            ctx.enter_context(nc.semaphore("s_in0")),
            ctx.enter_context(nc.semaphore("s_in1")),
        ]
        s_out = [
            ctx.enter_context(nc.semaphore("s_out0")),
            ctx.enter_context(nc.semaphore("s_out1")),
        ]
        s_act = ctx.enter_context(nc.semaphore("s_act"))
        s_dve = ctx.enter_context(nc.semaphore("s_dve"))
        s_gp = ctx.enter_context(nc.semaphore("s_gp"))

        # ---- two-pass schedule: pass 1 records per-op sem values (marks),
        # ---- pass 2 emits with waits resolved from the marks.
        marks = {}

        def mk(engkey, name, t, ctr):
            marks[(engkey, name, t)] = ctr

        def sched_sp(sp):
            for i in range(n + 1):
                if i < n:
                    j = i % 2
                    if sp is not None:
                        if i >= 2:
                            sp.wait_ge(s_act, marks[("a", "zlin", i - 2)])
                            sp.wait_ge(s_dve, marks[("d", "wc", i - 2)])
                        sp.dma_start(out=zb[j][:], in_=zb_d[:, bass.ts(i, TK)]
                                     ).then_inc(s_in[j], 16)
                        sp.dma_start(out=ds[j][:], in_=ds_d[:, bass.ts(i, TK)]
                                     ).then_inc(s_in[j], 16)
                        sp.dma_start(out=pf[j][:], in_=pf_d[:, bass.ts(i, TK)]
                                     ).then_inc(s_in[j], 16)
                        sp.dma_start(out=col[j][:], in_=pc_d[:, bass.ts(i, TK * 3)]
                                     ).then_inc(s_in[j], 16)
                if i >= 1:
                    t = i - 1
                    if sp is not None:
                        sp.wait_ge(s_dve, marks[("d", "rgb", t)])
                        sp.wait_ge(s_act, marks[("a", "alpha", t)])
                        sp.dma_start(
                            out=out_d[:, bass.ts(t, T * 4)], in_=ot[t % 2][:]
                        ).then_inc(s_out[t % 2], 16)
            if sp is not None:
                sp.wait_ge(s_out[0], 16 * ((n + 1) // 2))
                sp.wait_ge(s_out[1], 16 * (n // 2))

        def sched_act(act):
            c = 0
            for i in range(n + 1):
                if i < n:
                    j = i % 2
                    if act is not None:
                        act.wait_ge(s_in[j], 64 * (i // 2 + 1))
                        if i >= 2:
                            act.wait_ge(s_dve, marks[("d", "wc", i - 2)])
                        act.activation(sig[j][:], ds[j][:], Act.Sigmoid,
                                       scale=-1.0 / SIGMA).then_inc(s_act, 1)
                    c += 1; mk("a", "sig", i, c)
                    if act is not None:
                        act.activation(
                            zlin[j][:], zb[j][:], Act.Copy,
                            bias=ZFAR / (ZFAR - ZNEAR),
                            scale=-1.0 / (ZFAR - ZNEAR),
                        ).then_inc(s_act, 1)
                    c += 1; mk("a", "zlin", i, c)
                    if act is not None:
                        act.wait_ge(s_dve, marks[("d", "prob", i)])
                        act.activation(lnq[:], sig[j][:], Act.Ln, bias=1.0,
                                       scale=-1.0).then_inc(s_act, 1)
                    c += 1; mk("a", "lnq", i, c)
                    if act is not None:
                        act.wait_ge(s_gp, marks[("g", "zd", i)])
                        act.activation(zd[:], zd[:], Act.Exp,
                                       scale=1.0 / GAMMA).then_inc(s_act, 1)
                    c += 1; mk("a", "ex", i, c)
                    if act is not None:
                        act.activation(
                            delta[i % 2][:], zmax[:], Act.Exp,
                            bias=EPS / GAMMA, scale=-1.0 / GAMMA,
                        ).then_inc(s_act, 1)
                    c += 1; mk("a", "delta", i, c)
                    if act is not None:
                        act.wait_ge(s_dve, marks[("d", "qsum", i)])
                        act.activation(prodq[i % 2][:], qsum[:], Act.Exp
                                       ).then_inc(s_act, 1)
                    c += 1; mk("a", "prodq", i, c)
                if i >= 1:
                    t = i - 1
                    if act is not None:
                        act.wait_ge(s_dve, marks[("d", "denom", t)])
                        act.activation(denom[:], denom[:], Act.Ln
                                       ).then_inc(s_act, 1)
                    c += 1; mk("a", "lnd", t, c)
                    if act is not None:
                        act.drain()
                        act.activation(rcp[:], denom[:], Act.Exp, scale=-1.0
                                       ).then_inc(s_act, 1)
                    c += 1; mk("a", "rcp", t, c)
                    if act is not None:
                        if t >= 2:
                            act.wait_ge(s_out[t % 2], 16 * ((t - 2) // 2 + 1))
                        ot_v = ot[t % 2][:].rearrange("p (t q) -> p t q", q=4)
                        act.activation(
                            ot_v[:, :, 3:4], prodq[t % 2][:].unsqueeze(2),
                            Act.Copy, bias=1.0, scale=-1.0,
                        ).then_inc(s_act, 1)
                    c += 1; mk("a", "alpha", t, c)

        def sched_dve(dve):
            c = 0
            for i in range(n + 1):
                if i < n:
                    j = i % 2
                    if dve is not None:
                        dve.wait_ge(s_in[j], 64 * (i // 2 + 1))
                        dve.wait_ge(s_act, marks[("a", "zlin", i)])
                        dve.scalar_tensor_tensor(
                            out=sig[j][:], in0=pf[j][:], scalar=0.0, in1=sig[j][:],
                            op0=Alu.is_ge, op1=Alu.mult,
                        ).then_inc(s_dve, 1)
                        dve.drain()
                    c += 1; mk("d", "prob", i, c)
                    if dve is not None:
                        dve.scalar_tensor_tensor(
                            out=zlin[j][:], in0=pf[j][:], scalar=0.0, in1=zlin[j][:],
                            op0=Alu.is_ge, op1=Alu.mult,
                        ).then_inc(s_dve, 1)
                        dve.drain()
                    c += 1; mk("d", "zinv", i, c)
                    if dve is not None:
                        dve.tensor_reduce(
                            out=zmax[:],
                            in_=zlin[j][:].rearrange("p (t k) -> p t k", k=K),
                            op=Alu.max, axis=Ax.X,
                        ).then_inc(s_dve, 1)
                    c += 1; mk("d", "zmax", i, c)
                    if dve is not None:
                        dve.wait_ge(s_act, marks[("a", "lnq", i)])
                        dve.tensor_reduce(
                            out=qsum[:],
                            in_=lnq[:].rearrange("p (t k) -> p t k", k=K),
                            op=Alu.add, axis=Ax.X,
                        ).then_inc(s_dve, 1)
                    c += 1; mk("d", "qsum", i, c)
                if i >= 1:
                    t = i - 1
                    if dve is not None:
                        dve.wait_ge(s_gp, marks[("g", "w", t)])
                        dve.tensor_reduce(
                            out=wsum[:],
                            in_=sig[t % 2][:].rearrange("p (t k) -> p t k", k=K),
                            op=Alu.add, axis=Ax.X,
                        ).then_inc(s_dve, 1)
                        dve.drain()
                    c += 1; mk("d", "wsum", t, c)
                    if dve is not None:
                        dve.wait_ge(s_act, marks[("a", "delta", t)])
                        dve.tensor_tensor(
                            out=denom[:], in0=wsum[:], in1=delta[t % 2][:],
                            op=Alu.add,
                        ).then_inc(s_dve, 1)
                    c += 1; mk("d", "denom", t, c)
                    if dve is not None:
                        wc_v = wc[:].rearrange("p (t c k) -> p t c k", c=3, k=K)
                        dve.tensor_tensor(
                            out=wc_v,
                            in0=sig[t % 2][:].rearrange("p (t k) -> p t k", k=K)
                                .unsqueeze(2).broadcast_to((P, T, 3, K)),
                            in1=col[t % 2][:].rearrange(
                                "p (t k c) -> p t c k", k=K, c=3),
                            op=Alu.mult,
                        ).then_inc(s_dve, 1)
                        dve.drain()
                    c += 1; mk("d", "wc", t, c)
                    if dve is not None:
                        csum_v = csum[:].rearrange("p (t c) -> p t c", c=3)
                        dve.tensor_reduce(
                            out=csum_v, in_=wc_v, op=Alu.add, axis=Ax.X
                        ).then_inc(s_dve, 1)
                        dve.drain()
                    c += 1; mk("d", "csum", t, c)
                    if dve is not None:
                        csum_v = csum[:].rearrange("p (t c) -> p t c", c=3)
                        dve.tensor_tensor(
                            out=csum_v, in0=csum_v,
                            in1=delta[t % 2][:].unsqueeze(2).broadcast_to((P, T, 3)),
                            op=Alu.add,
                        ).then_inc(s_dve, 1)
                        dve.drain()
                    c += 1; mk("d", "t3", t, c)
                    if dve is not None:
                        if t >= 2:
                            dve.wait_ge(s_out[t % 2], 16 * ((t - 2) // 2 + 1))
                        dve.wait_ge(s_act, marks[("a", "rcp", t)])
                        ot_v = ot[t % 2][:].rearrange("p (t q) -> p t q", q=4)
                        dve.tensor_tensor(
                            out=ot_v[:, :, 0:3],
                            in0=csum[:].rearrange("p (t c) -> p t c", c=3),
                            in1=rcp[:].unsqueeze(2).broadcast_to((P, T, 3)),
                            op=Alu.mult,
                        ).then_inc(s_dve, 1)
                    c += 1; mk("d", "rgb", t, c)

        def sched_gp(gp):
            c = 0
            for i in range(n):
                j = i % 2
                if gp is not None:
                    gp.wait_ge(s_dve, marks[("d", "zmax", i)])
                    gp.tensor_tensor(
                        out=zd[:].rearrange("p (t k) -> p t k", k=K),
                        in0=zlin[j][:].rearrange("p (t k) -> p t k", k=K),
                        in1=zmax[:].unsqueeze(2).broadcast_to((P, T, K)),
                        op=Alu.subtract,
                    ).then_inc(s_gp, 1)
                c += 1; mk("g", "zd", i, c)
                if gp is not None:
                    gp.wait_ge(s_act, marks[("a", "ex", i)])
                    gp.tensor_tensor(
                        out=sig[j][:], in0=sig[j][:], in1=zd[:], op=Alu.mult
                    ).then_inc(s_gp, 1)
                    gp.drain()
                c += 1; mk("g", "w", i, c)

        # pass 1: record marks
        sched_sp(None)
        sched_act(None)
        sched_dve(None)
        sched_gp(None)

        blk = ctx.enter_context(nc.Block())

        @blk.sync
        def _(sp):
            sched_sp(sp)

        @blk.scalar
        def _(act):
            sched_act(act)

        @blk.vector
        def _(dve):
            sched_dve(dve)

        @blk.gpsimd
        def _(gp):
            sched_gp(gp)

    return nc


_CACHE = {}


def _get_program(rows=2048, T=256):
    key = (rows, T)
    if key not in _CACHE:
        _CACHE[key] = build_program(rows, T)
    return _CACHE[key]


def _run(pixel_colors, zbuf, dists, pix_to_face, trace=False):
    from concourse.bass_utils import run_bass_kernel_spmd

    N, H, W, Kk = zbuf.shape
    assert (N, H, W, Kk) == (8, 512, 512, 8), (N, H, W, Kk)
    rows = H * W // P  # 2048

    nc = _get_program(rows=rows, T=256)

    pc = np.ascontiguousarray(np.asarray(pixel_colors, dtype=np.float32))
    zb = np.ascontiguousarray(np.asarray(zbuf, dtype=np.float32))
    ds = np.ascontiguousarray(np.asarray(dists, dtype=np.float32))
    pf = np.ascontiguousarray(np.asarray(pix_to_face, dtype=np.int32))

    in_maps = []
    for i in range(N_CORES):
        in_maps.append(
            {
                "zbuf": zb[i].reshape(P, rows * K),
                "dists": ds[i].reshape(P, rows * K),
                "pix_to_face": pf[i].reshape(P, rows * K),
                "pixel_colors": pc[i].reshape(P, rows * K * 3),
            }
        )

    res = run_bass_kernel_spmd(
        nc, in_maps, core_ids=list(range(N_CORES)), trace=trace
    )
    out = np.stack(
        [res.results[i]["out"].reshape(H, W, 4) for i in range(N_CORES)], axis=0
    )
    return out, res


def kernel(pixel_colors, zbuf, dists, pix_to_face):
    out, _ = _run(pixel_colors, zbuf, dists, pix_to_face, trace=False)
    return out
